# revision 1
# baseline (speedup 1.0000x reference)
"""Trainium2 Bass kernel for CRF mean-field iteration (nn_CRF).

Math (derived from the reference):
    comp = -I  =>  each iteration is   x <- x0 + w * smooth(softmax(x, C))
    output = log_softmax(x_final, C)
where smooth = per-channel separable 11-tap Gaussian blur over H then W
('same' zero padding, center tap zeroed, per-sample spacing).

Strategy (per core, 2 samples, pure data parallel over batch):
  - State layout in SBUF: xbuf[p, c, j, w] = x[c, 128*j + p, w]
    (h on partitions in 3 chunks of 128; free dim = (channel, chunk, width),
    so per-channel and whole-sample DMA views stay 3-dim contiguous).
  - Softmax: ACT exp (in-place), channel-sum via a GPSIMD pairwise tree,
    r = 1/S via the fast DVE Newton reciprocal, p = e*r as per-channel
    contiguous DVE multiplies.
  - Conv along H as matmul with the *data as the stationary operand*
    (out1[w,h'] = sum_h p[h,w]*Th[h,h']), which lands transposed in PSUM.
    Conv along W the same way on out1, landing back in [h', w'] layout.
    Th/Tw are banded symmetric Toeplitz matrices built on the host from the
    runtime spacing/theta inputs; smoothness_weight is folded into Tw.
    Band structure: for contraction chunk j only output cols
    [128j-5, 128j+133) are touched; PSUM has_written semantics handle the
    overlap (accumulate where written, overwrite where not).
  - x_new = x0 + s fused as one DVE tensor_add reading PSUM directly.
"""

import sys

if "/opt/trn_rl_repo" not in sys.path:
    sys.path.insert(0, "/opt/trn_rl_repo")

from contextlib import ExitStack

import numpy as np

import concourse.bass as bass
import concourse.tile as tile
from concourse import bacc, mybir

F32 = mybir.dt.float32
AF = mybir.ActivationFunctionType

B, C, H, W = 16, 16, 384, 384
N_CORES = 8
BPC = B // N_CORES  # samples per core
N_ITER = 5
FS = 11
HALF = FS // 2  # 5
P = 128
NCH = H // P  # 3 h-chunks
NCW = W // P  # 3 w-chunks


def _band(j, n):
    """Output-column range touched by contraction chunk j of a banded T."""
    return max(0, P * j - HALF), min(n, P * j + P + HALF)


def _crf_kernel(ctx, tc, out_d, x_in, th_in, tw_in, n_samples, n_iter, full_j0):
    nc = tc.nc

    state = ctx.enter_context(tc.tile_pool(name="state", bufs=1))
    mats = ctx.enter_context(tc.tile_pool(name="mats", bufs=1))
    stage = ctx.enter_context(tc.tile_pool(name="stage", bufs=2))
    small = ctx.enter_context(tc.tile_pool(name="small", bufs=1))
    psum = ctx.enter_context(tc.tile_pool(name="psum", bufs=2, space="PSUM"))

    xbuf = state.tile([P, C, NCH, W], F32, tag="xbuf")
    x0buf = state.tile([P, C, NCH, W], F32, tag="x0buf")

    for b in range(n_samples):
        # ---- load inputs for this sample ----
        # One DMA for the whole sample: fewer HWDGE-queue semaphores for
        # downstream waits (TT sync-wait ISA limit) and better DMA batching.
        nc.sync.dma_start(
            out=x0buf[:],
            in_=x_in[b].rearrange("c (j p) w -> p c j w", p=P),
        )
        th_sb = mats.tile([P, NCH, H], F32, tag="th")
        tw_sb = mats.tile([P, NCW, W], F32, tag="tw")
        nc.sync.dma_start(out=th_sb[:], in_=th_in[b].rearrange("(j p) n -> p j n", p=P))
        nc.sync.dma_start(out=tw_sb[:], in_=tw_in[b].rearrange("(j p) n -> p j n", p=P))

        # Softmax emission helpers. Emitted interleaved with the previous
        # iteration's conv loop so each engine's program order (~= Tile
        # schedule order) lets exps/partial-sums run DURING the conv phase.
        def emit_exp_cg(src, vts, cg):
            sl = slice(4 * cg, 4 * cg + 4)
            for j in range(NCH):
                nc.scalar.activation(
                    out=xbuf[:, sl, j], in_=src[:, sl, j], func=AF.Exp
                )
                ut = small.tile([P, 2, W], F32, tag="tu")
                nc.gpsimd.tensor_add(
                    ut[:], xbuf[:, 4 * cg : 4 * cg + 2, j],
                    xbuf[:, 4 * cg + 2 : 4 * cg + 4, j],
                )
                nc.vector.tensor_add(
                    vts[j][:, cg : cg + 1], ut[:, 0:1], ut[:, 1:2]
                )

        def emit_s_and_p(vts, sball, rall):
            for j in range(NCH):
                wt = small.tile([P, 2, W], F32, tag="twv")
                nc.gpsimd.tensor_add(wt[:, 0:1], vts[j][:, 0:1], vts[j][:, 1:2])
                nc.gpsimd.tensor_add(wt[:, 1:2], vts[j][:, 2:3], vts[j][:, 3:4])
                nc.vector.tensor_add(
                    sball[:, j : j + 1], wt[:, 0:1], wt[:, 1:2]
                )
                nc.vector.reciprocal_approx_fast(rall[:, j], sball[:, j])
            for c in range(C):
                nc.vector.tensor_mul(out=xbuf[:, c], in0=xbuf[:, c], in1=rall[:])

        def new_smax_tiles():
            sball = small.tile([P, NCH, W], F32, tag="S")
            rall = small.tile([P, NCH, W], F32, tag="r")
            vts = [small.tile([P, 4, W], F32, tag=f"tv{j}", name=f"vt{j}") for j in range(NCH)]
            return sball, rall, vts

        # Prologue: softmax of iteration 0 from x0.
        sball, rall, vts = new_smax_tiles()
        for cg in range(4):
            emit_exp_cg(x0buf, vts, cg)
        emit_s_and_p(vts, sball, rall)

        for it in range(n_iter):
            last = it == n_iter - 1
            if not last:
                nball, nrall, nvts = new_smax_tiles()
            # ---- smoothing convs + fused x-update, per channel ----
            for c in range(C):
                pA = psum.tile([P, NCH, 512], F32, tag="ps")
                for m in range(NCW):
                    for j in range(NCH):
                        # CoreSim needs j==0 to cover the full width (its
                        # pending-zero model can't mix accumulate/overwrite in
                        # one matmul); HW has_written handles the banded
                        # overlap per element, so skip the extra columns there.
                        n0, n1 = (0, H) if (j == 0 and full_j0) else _band(j, H)
                        nc.tensor.matmul(
                            pA[:, m, n0:n1],
                            lhsT=xbuf[:, c, j, m * P : (m + 1) * P],
                            rhs=th_sb[:, j, n0:n1],
                            start=(j == 0),
                            stop=(j == NCH - 1),
                        )
                o1 = stage.tile([P, NCW, H], F32, tag="o1")
                nc.scalar.copy(out=o1[:], in_=pA[:, :, 0:H])
                pB = psum.tile([P, NCH, 512], F32, tag="ps")
                for m in range(NCH):
                    for j in range(NCW):
                        n0, n1 = (0, W) if (j == 0 and full_j0) else _band(j, W)
                        nc.tensor.matmul(
                            pB[:, m, n0:n1],
                            lhsT=o1[:, j, m * P : (m + 1) * P],
                            rhs=tw_sb[:, j, n0:n1],
                            start=(j == 0),
                            stop=(j == NCW - 1),
                        )
                nc.vector.tensor_add(
                    out=xbuf[:, c], in0=x0buf[:, c], in1=pB[:, :, 0:W]
                )
                # Next iteration's softmax for this channel group becomes
                # ready as soon as its 4 channels' updates land — emit here
                # so it overlaps the remaining channels' convs.
                if not last and c % 4 == 3:
                    emit_exp_cg(xbuf, nvts, c // 4)
            if not last:
                emit_s_and_p(nvts, nball, nrall)
                sball, rall, vts = nball, nrall, nvts

        # ---- final log_softmax: out = x - log(sum_c exp(x)) ----
        # Dedicated exp scratch: reusing x0buf here made the NEXT sample's x0
        # DMA wait for the whole final pass (measured 130us PE stall).
        lball = small.tile([P, NCH, W], F32, tag="r")
        for j in range(NCH):
            vt = small.tile([P, 4, W], F32, tag="tv")
            for cg in range(4):
                sl = slice(4 * cg, 4 * cg + 4)
                fe = stage.tile([P, 4, W], F32, tag="o1")
                nc.scalar.activation(
                    out=fe[:], in_=xbuf[:, sl, j], func=AF.Exp
                )
                ut = small.tile([P, 2, W], F32, tag="tu")
                nc.gpsimd.tensor_add(ut[:], fe[:, 0:2], fe[:, 2:4])
                nc.vector.tensor_add(vt[:, cg : cg + 1], ut[:, 0:1], ut[:, 1:2])
            wt = small.tile([P, 2, W], F32, tag="twv")
            nc.gpsimd.tensor_add(wt[:, 0:1], vt[:, 0:1], vt[:, 1:2])
            nc.gpsimd.tensor_add(wt[:, 1:2], vt[:, 2:3], vt[:, 3:4])
            sb = small.tile([P, 1, W], F32, tag="S")
            nc.vector.tensor_add(sb[:], wt[:, 0:1], wt[:, 1:2])
            nc.scalar.activation(out=lball[:, j], in_=sb[:, 0], func=AF.Ln)
        for c in range(C):
            nc.vector.tensor_sub(out=xbuf[:, c], in0=xbuf[:, c], in1=lball[:])
        nc.sync.dma_start(
            out=out_d[b].rearrange("c (j p) w -> p c j w", p=P),
            in_=xbuf[:],
        )


def build_nc(n_samples=BPC, n_iter=N_ITER, full_j0=False):
    # Bacc (not plain Bass): its compile() pass legalizes multi-wait
    # instructions via InstEventSemaphore — walrus caps regular instructions
    # at ONE sync wait.
    nc = bacc.Bacc()
    x_in = nc.dram_tensor("x", [n_samples, C, H, W], F32, kind="ExternalInput")
    th_in = nc.dram_tensor("th", [n_samples, H, H], F32, kind="ExternalInput")
    tw_in = nc.dram_tensor("tw", [n_samples, W, W], F32, kind="ExternalInput")
    out_d = nc.dram_tensor("out", [n_samples, C, H, W], F32, kind="ExternalOutput")
    with tile.TileContext(nc) as tc:
        with ExitStack() as ctx:
            _crf_kernel(ctx, tc, out_d, x_in, th_in, tw_in, n_samples, n_iter, full_j0)
    nc.finalize()
    return nc


def make_toeplitz(spacing, inv_theta, size, weight=1.0):
    """Banded symmetric Toeplitz matrix for the 1D 'same' correlation."""
    d = spacing * np.arange(-(FS // 2), FS // 2 + 1, dtype=np.float32)
    k = np.exp(-((d * inv_theta) ** 2) / 2.0).astype(np.float32)
    k[FS // 2] = 0.0
    t = np.zeros((size, size), dtype=np.float32)
    for tap in range(FS):
        off = tap - FS // 2  # out[h] += k[tap] * x[h + off]
        idx = np.arange(max(0, -off), min(size, size - off))
        t[idx + off, idx] = k[tap]
    return (t * weight).astype(np.float32)


def host_prep(x, spatial_spacings, smoothness_weight, inv_smoothness_theta):
    """Build per-sample Th (H-conv) and weight-scaled Tw (W-conv) matrices."""
    w = float(np.asarray(smoothness_weight))
    th = np.stack(
        [
            make_toeplitz(float(spatial_spacings[b, 0]), float(inv_smoothness_theta[0]), H)
            for b in range(x.shape[0])
        ]
    )
    tw = np.stack(
        [
            make_toeplitz(
                float(spatial_spacings[b, 1]), float(inv_smoothness_theta[1]), W, weight=w
            )
            for b in range(x.shape[0])
        ]
    )
    return th, tw


_NC_CACHE = {}


def kernel(x, spatial_spacings, smoothness_weight, inv_smoothness_theta):
    from concourse.bass_utils import run_bass_kernel_spmd

    x = np.ascontiguousarray(np.asarray(x), dtype=np.float32)
    spatial_spacings = np.asarray(spatial_spacings, dtype=np.float32)
    th, tw = host_prep(x, spatial_spacings, smoothness_weight, inv_smoothness_theta)

    key = (BPC, N_ITER)
    if key not in _NC_CACHE:
        _NC_CACHE[key] = build_nc(BPC, N_ITER)
    nc = _NC_CACHE[key]

    core_ids = list(range(N_CORES))
    in_maps = []
    for i in core_ids:
        sl = slice(i * BPC, (i + 1) * BPC)
        in_maps.append({"x": x[sl], "th": th[sl], "tw": tw[sl]})
    res = run_bass_kernel_spmd(nc, in_maps, core_ids)
    out = np.concatenate([res.results[i]["out"] for i in core_ids], axis=0)
    return out.astype(np.float32)


if __name__ == "__main__":
    rng = np.random.default_rng(0)
    x = rng.standard_normal((B, C, H, W), dtype=np.float32)
    out = kernel(
        x,
        np.ones((B, 2), np.float32),
        np.float32(1.0),
        np.ones((2,), np.float32),
    )
    print(out.shape, out.dtype)



# revision 5
# speedup vs baseline: 1.3028x; 1.3028x over previous
"""Trainium2 Bass kernel for CRF mean-field iteration (nn_CRF).

Math (derived from the reference):
    comp = -I  =>  each iteration is   x <- x0 + w * smooth(softmax(x, C))
    output = log_softmax(x_final, C)
where smooth = per-channel separable 11-tap Gaussian blur over H then W
('same' zero padding, center tap zeroed, per-sample spacing).

Strategy (per core, 2 samples, pure data parallel over batch):
  - State layout in SBUF: xbuf[p, c, j, w] = x[c, 128*j + p, w]
    (h on partitions in 3 chunks of 128; free dim = (channel, chunk, width),
    so per-channel and whole-sample DMA views stay 3-dim contiguous).
  - The correctness gate is rel_err < 2e-2; everything in the p/conv path
    runs in bf16 (measured end-to-end ~4e-4 in a host simulation):
    exp -> bf16 e, bf16 tree sums, f32 fast reciprocal, bf16 p = e*r,
    bf16 matmuls (PSUM accumulates f32), bf16 o1 staging.
  - Softmax: ACT exp (xbuf f32 -> ebuf bf16), channel-sum via a
    GPSIMD/DVE pairwise tree, r = 1/S via the fast DVE Newton reciprocal,
    p = e*r as per-channel contiguous bf16 DVE multiplies (2x packed mode).
  - Conv along H as matmul with the *data as the stationary operand*
    (out1[w,h'] = sum_h p[h,w]*Th[h,h']), which lands transposed in PSUM.
    Conv along W the same way on out1, landing back in [h', w'] layout.
    Th/Tw are banded symmetric Toeplitz matrices built on the host from the
    runtime spacing/theta inputs; smoothness_weight is folded into Tw.
    Band structure: for contraction chunk j only output cols
    [128j-5, 128j+133) are touched; PSUM has_written semantics handle the
    overlap (accumulate where written, overwrite where not).
  - x_new = x0(bf16) + s fused as one DVE tensor_add reading PSUM directly,
    writing the f32 master xbuf.
"""

import sys

if "/opt/trn_rl_repo" not in sys.path:
    sys.path.insert(0, "/opt/trn_rl_repo")

from contextlib import ExitStack

import numpy as np

import concourse.bass as bass
import concourse.tile as tile
from concourse import bacc, mybir

F32 = mybir.dt.float32
BF16 = mybir.dt.bfloat16
AF = mybir.ActivationFunctionType

B, C, H, W = 16, 16, 384, 384
N_CORES = 8
BPC = B // N_CORES  # samples per core
N_ITER = 5
FS = 11
HALF = FS // 2  # 5
P = 128
NCH = H // P  # 3 h-chunks
NCW = W // P  # 3 w-chunks


def _band(j, n):
    """Output-column range touched by contraction chunk j of a banded T."""
    return max(0, P * j - HALF), min(n, P * j + P + HALF)


def _crf_kernel(ctx, tc, out_d, x_in, th_in, tw_in, n_samples, n_iter, full_j0):
    nc = tc.nc

    state = ctx.enter_context(tc.tile_pool(name="state", bufs=1))
    mats = ctx.enter_context(tc.tile_pool(name="mats", bufs=1))
    stage = ctx.enter_context(tc.tile_pool(name="stage", bufs=2))
    smax = ctx.enter_context(tc.tile_pool(name="smax", bufs=1))
    small = ctx.enter_context(tc.tile_pool(name="small", bufs=3))
    psum = ctx.enter_context(tc.tile_pool(name="psum", bufs=2, space="PSUM"))

    xbuf = state.tile([P, C, NCH, W], F32, tag="xbuf")
    x0b = state.tile([P, C, NCH, W], BF16, tag="x0b")
    ebuf = state.tile([P, C, NCH, W], BF16, tag="ebuf")

    for b in range(n_samples):
        # ---- load inputs for this sample ----
        nc.sync.dma_start(
            out=x0b[:],
            in_=x_in[b].rearrange("c (j p) w -> p c j w", p=P),
        )
        th_sb = mats.tile([P, NCH, H], BF16, tag="th")
        tw_sb = mats.tile([P, NCW, W], BF16, tag="tw")
        nc.sync.dma_start(out=th_sb[:], in_=th_in[b].rearrange("(j p) n -> p j n", p=P))
        nc.sync.dma_start(out=tw_sb[:], in_=tw_in[b].rearrange("(j p) n -> p j n", p=P))

        # Softmax emission helpers. Emitted interleaved with the previous
        # iteration's conv loop so each engine's program order (~= Tile
        # schedule order) lets exps/partial-sums run DURING the conv phase.
        def emit_exp_cg(src, vts, cg):
            sl = slice(4 * cg, 4 * cg + 4)
            for j in range(NCH):
                nc.scalar.activation(
                    out=ebuf[:, sl, j], in_=src[:, sl, j], func=AF.Exp
                )
                ut = small.tile([P, 2, W], BF16, tag="tu")
                nc.gpsimd.tensor_add(
                    ut[:], ebuf[:, 4 * cg : 4 * cg + 2, j],
                    ebuf[:, 4 * cg + 2 : 4 * cg + 4, j],
                )
                nc.vector.tensor_add(
                    vts[j][:, cg : cg + 1], ut[:, 0:1], ut[:, 1:2]
                )

        def emit_s_and_p(vts, sball, rall, rb):
            for j in range(NCH):
                wt = small.tile([P, 2, W], BF16, tag="twv")
                nc.gpsimd.tensor_add(wt[:, 0:1], vts[j][:, 0:1], vts[j][:, 1:2])
                nc.gpsimd.tensor_add(wt[:, 1:2], vts[j][:, 2:3], vts[j][:, 3:4])
                nc.vector.tensor_add(
                    sball[:, j : j + 1], wt[:, 0:1], wt[:, 1:2]
                )
                nc.vector.reciprocal_approx_fast(rall[:, j], sball[:, j])
            nc.vector.tensor_scalar_mul(rb[:], rall[:], 1.0)
            for c in range(C):
                nc.vector.tensor_mul(out=ebuf[:, c], in0=ebuf[:, c], in1=rb[:])

        def new_smax_tiles():
            sball = smax.tile([P, NCH, W], F32, tag="S")
            rall = smax.tile([P, NCH, W], F32, tag="r")
            rb = smax.tile([P, NCH, W], BF16, tag="rb")
            vts = [smax.tile([P, 4, W], BF16, tag=f"tv{j}", name=f"vt{j}") for j in range(NCH)]
            return sball, rall, rb, vts

        # Prologue: softmax of iteration 0 from x0.
        sball, rall, rb, vts = new_smax_tiles()
        for cg in range(4):
            emit_exp_cg(x0b, vts, cg)
        emit_s_and_p(vts, sball, rall, rb)

        for it in range(n_iter):
            last = it == n_iter - 1
            if not last:
                nball, nrall, nrb, nvts = new_smax_tiles()
            # ---- smoothing convs + fused x-update, per channel ----
            for c in range(C):
                pA = psum.tile([P, NCH, 512], F32, tag="ps")
                for m in range(NCW):
                    for j in range(NCH):
                        # CoreSim needs j==0 to cover the full width (its
                        # pending-zero model can't mix accumulate/overwrite in
                        # one matmul); HW has_written handles the banded
                        # overlap per element, so skip the extra columns there.
                        n0, n1 = (0, H) if (j == 0 and full_j0) else _band(j, H)
                        nc.tensor.matmul(
                            pA[:, m, n0:n1],
                            lhsT=ebuf[:, c, j, m * P : (m + 1) * P],
                            rhs=th_sb[:, j, n0:n1],
                            start=(j == 0),
                            stop=(j == NCH - 1),
                        )
                o1 = stage.tile([P, NCW, H], BF16, tag="o1")
                nc.scalar.copy(out=o1[:], in_=pA[:, :, 0:H])
                pB = psum.tile([P, NCH, 512], F32, tag="ps")
                for m in range(NCH):
                    for j in range(NCW):
                        n0, n1 = (0, W) if (j == 0 and full_j0) else _band(j, W)
                        nc.tensor.matmul(
                            pB[:, m, n0:n1],
                            lhsT=o1[:, j, m * P : (m + 1) * P],
                            rhs=tw_sb[:, j, n0:n1],
                            start=(j == 0),
                            stop=(j == NCW - 1),
                        )
                nc.vector.tensor_add(
                    out=xbuf[:, c], in0=x0b[:, c], in1=pB[:, :, 0:W]
                )
                # Next iteration's softmax for this channel group becomes
                # ready as soon as its 4 channels' updates land — emit here
                # so it overlaps the remaining channels' convs.
                if not last and c % 4 == 3:
                    emit_exp_cg(xbuf, nvts, c // 4)
            if not last:
                emit_s_and_p(nvts, nball, nrall, nrb)
                sball, rall, rb, vts = nball, nrall, nrb, nvts

        # ---- final log_softmax: out = x - log(sum_c exp(x)) ----
        lball = smax.tile([P, NCH, W], F32, tag="r")
        for j in range(NCH):
            vt = small.tile([P, 4, W], BF16, tag="tv")
            for cg in range(4):
                sl = slice(4 * cg, 4 * cg + 4)
                nc.scalar.activation(
                    out=ebuf[:, sl, j], in_=xbuf[:, sl, j], func=AF.Exp
                )
                ut = small.tile([P, 2, W], BF16, tag="tu")
                nc.gpsimd.tensor_add(
                    ut[:], ebuf[:, 4 * cg : 4 * cg + 2, j],
                    ebuf[:, 4 * cg + 2 : 4 * cg + 4, j],
                )
                nc.vector.tensor_add(vt[:, cg : cg + 1], ut[:, 0:1], ut[:, 1:2])
            wt = small.tile([P, 2, W], BF16, tag="twv")
            nc.gpsimd.tensor_add(wt[:, 0:1], vt[:, 0:1], vt[:, 1:2])
            nc.gpsimd.tensor_add(wt[:, 1:2], vt[:, 2:3], vt[:, 3:4])
            sb = small.tile([P, 1, W], F32, tag="Sf")
            nc.vector.tensor_add(sb[:], wt[:, 0:1], wt[:, 1:2])
            nc.scalar.activation(out=lball[:, j], in_=sb[:, 0], func=AF.Ln)
        for c in range(C):
            eng = nc.vector if c % 2 == 0 else nc.gpsimd
            eng.tensor_sub(out=xbuf[:, c], in0=xbuf[:, c], in1=lball[:])
        nc.sync.dma_start(
            out=out_d[b].rearrange("c (j p) w -> p c j w", p=P),
            in_=xbuf[:],
        )


def build_nc(n_samples=BPC, n_iter=N_ITER, full_j0=False):
    # Bacc (not plain Bass): its compile() pass legalizes multi-wait
    # instructions via InstEventSemaphore — walrus caps regular instructions
    # at ONE sync wait.
    nc = bacc.Bacc()
    x_in = nc.dram_tensor("x", [n_samples, C, H, W], BF16, kind="ExternalInput")
    th_in = nc.dram_tensor("th", [n_samples, H, H], BF16, kind="ExternalInput")
    tw_in = nc.dram_tensor("tw", [n_samples, W, W], BF16, kind="ExternalInput")
    out_d = nc.dram_tensor("out", [n_samples, C, H, W], F32, kind="ExternalOutput")
    with tile.TileContext(nc) as tc:
        with ExitStack() as ctx:
            _crf_kernel(ctx, tc, out_d, x_in, th_in, tw_in, n_samples, n_iter, full_j0)
    nc.finalize()
    return nc


def make_toeplitz(spacing, inv_theta, size, weight=1.0):
    """Banded symmetric Toeplitz matrix for the 1D 'same' correlation."""
    d = spacing * np.arange(-(FS // 2), FS // 2 + 1, dtype=np.float32)
    k = np.exp(-((d * inv_theta) ** 2) / 2.0).astype(np.float32)
    k[FS // 2] = 0.0
    t = np.zeros((size, size), dtype=np.float32)
    for tap in range(FS):
        off = tap - FS // 2  # out[h] += k[tap] * x[h + off]
        idx = np.arange(max(0, -off), min(size, size - off))
        t[idx + off, idx] = k[tap]
    return (t * weight).astype(np.float32)


def host_prep(x, spatial_spacings, smoothness_weight, inv_smoothness_theta):
    """Build per-sample Th (H-conv) and weight-scaled Tw (W-conv) matrices
    plus the bf16 copy of x; all conv-path operands ship as bf16."""
    import ml_dtypes

    w = float(np.asarray(smoothness_weight))
    th = np.stack(
        [
            make_toeplitz(float(spatial_spacings[b, 0]), float(inv_smoothness_theta[0]), H)
            for b in range(x.shape[0])
        ]
    ).astype(ml_dtypes.bfloat16)
    tw = np.stack(
        [
            make_toeplitz(
                float(spatial_spacings[b, 1]), float(inv_smoothness_theta[1]), W, weight=w
            )
            for b in range(x.shape[0])
        ]
    ).astype(ml_dtypes.bfloat16)
    xb = np.ascontiguousarray(x).astype(ml_dtypes.bfloat16)
    return xb, th, tw


_NC_CACHE = {}


def kernel(x, spatial_spacings, smoothness_weight, inv_smoothness_theta):
    from concourse.bass_utils import run_bass_kernel_spmd

    x = np.ascontiguousarray(np.asarray(x), dtype=np.float32)
    spatial_spacings = np.asarray(spatial_spacings, dtype=np.float32)
    xb, th, tw = host_prep(x, spatial_spacings, smoothness_weight, inv_smoothness_theta)

    key = (BPC, N_ITER)
    if key not in _NC_CACHE:
        _NC_CACHE[key] = build_nc(BPC, N_ITER)
    nc = _NC_CACHE[key]

    core_ids = list(range(N_CORES))
    in_maps = []
    for i in core_ids:
        sl = slice(i * BPC, (i + 1) * BPC)
        in_maps.append({"x": xb[sl], "th": th[sl], "tw": tw[sl]})
    res = run_bass_kernel_spmd(nc, in_maps, core_ids)
    out = np.concatenate([res.results[i]["out"] for i in core_ids], axis=0)
    return out.astype(np.float32)


if __name__ == "__main__":
    rng = np.random.default_rng(0)
    x = rng.standard_normal((B, C, H, W), dtype=np.float32)
    out = kernel(
        x,
        np.ones((B, 2), np.float32),
        np.float32(1.0),
        np.ones((2,), np.float32),
    )
    print(out.shape, out.dtype)


# revision 11
# speedup vs baseline: 1.8912x; 1.4517x over previous
"""Trainium2 Bass kernel for CRF mean-field iteration (nn_CRF).

Math (derived from the reference):
    comp = -I  =>  each iteration is   x <- x0 + w * smooth(softmax(x, C))
    output = log_softmax(x_final, C)
where smooth = per-channel separable 11-tap Gaussian blur over H then W
('same' zero padding, center tap zeroed, per-sample spacing).

Strategy (per core, 2 samples, pure data parallel over batch). The
correctness gate is rel_err < 2e-2, so the whole p/conv path runs in bf16
(measured ~2.5e-3 end-to-end on HW); PSUM accumulates f32.

Per iteration (its 0..3):
  - exp: ScalarE ACT reads x straight from PSUM (see below), writes bf16 e.
  - channel-sum: flat wide bf16 adds on DVE (2x packed mode) with one
    4-channel group on GpSimd; 1/S via fast DVE Newton reciprocal (f32);
    p = e*r as flat bf16 DVE multiplies.
  - H-conv as matmul with the data stationary: out1[w,h'] = sum_h p[h,w]
    Th[h,h'] (banded Toeplitz moving operand, built on host) -> PSUM,
    drained to bf16 o1 by ScalarE/DVE (alternating channels).
  - W-conv: PSUM group is *seeded with x0* by an identity-stationary
    matmul (start=True streams x0b -> PSUM = x0), then the banded Tw
    matmuls accumulate on top, so PSUM ends holding x = x0 + s directly
    and no separate DVE x-update is needed; exp consumes it from PSUM.
Last iteration materializes x into SBUF f32 via a DVE add (x0 + s) for the
final log_softmax pass; output DMA'd as one block per sample.
"""

import sys

if "/opt/trn_rl_repo" not in sys.path:
    sys.path.insert(0, "/opt/trn_rl_repo")

from contextlib import ExitStack

import numpy as np

import concourse.bass as bass
import concourse.tile as tile
from concourse import bacc, mybir

F32 = mybir.dt.float32
BF16 = mybir.dt.bfloat16
AF = mybir.ActivationFunctionType

B, C, H, W = 16, 16, 384, 384
N_CORES = 8
BPC = B // N_CORES  # samples per core
N_ITER = 5
FS = 11
HALF = FS // 2  # 5
P = 128
NCH = H // P  # 3 h-chunks
NCW = W // P  # 3 w-chunks
NW = NCH * W  # flattened (h-chunk, w) free size


def _band(j, n):
    """Output-column range touched by contraction chunk j of a banded T."""
    return max(0, P * j - HALF), min(n, P * j + P + HALF)


def _f2(ap):
    return ap.rearrange("p a b -> p (a b)")


def _f3(ap):
    return ap.rearrange("p a b c -> p (a b c)")


def _crf_kernel(ctx, tc, out_d, x_in, th_in, tw_in, id_in, n_samples, n_iter, full_j0):
    nc = tc.nc

    state = ctx.enter_context(tc.tile_pool(name="state", bufs=1))
    mats = ctx.enter_context(tc.tile_pool(name="mats", bufs=1))
    stage = ctx.enter_context(tc.tile_pool(name="stage", bufs=2))
    smax = ctx.enter_context(tc.tile_pool(name="smax", bufs=1))
    small = ctx.enter_context(tc.tile_pool(name="small", bufs=2))
    psum = ctx.enter_context(tc.tile_pool(name="psum", bufs=2, space="PSUM"))

    xbuf = state.tile([P, C, NCH, W], F32, tag="xbuf")
    x0b = state.tile([P, C, NCH, W], BF16, tag="x0b")
    ebuf = state.tile([P, C, NCH, W], BF16, tag="ebuf")
    ident = state.tile([P, P], BF16, tag="ident")
    nc.sync.dma_start(out=ident[:], in_=id_in[:])

    for b in range(n_samples):
        # ---- load inputs for this sample ----
        nc.sync.dma_start(
            out=x0b[:],
            in_=x_in[b].rearrange("c (j p) w -> p c j w", p=P),
        )
        th_sb = mats.tile([P, NCH, H], BF16, tag="th")
        tw_sb = mats.tile([P, NCW, W], BF16, tag="tw")
        nc.sync.dma_start(out=th_sb[:], in_=th_in[b].rearrange("(j p) n -> p j n", p=P))
        nc.sync.dma_start(out=tw_sb[:], in_=tw_in[b].rearrange("(j p) n -> p j n", p=P))

        # --- softmax helpers (all flat 1D APs so DVE runs 2x packed bf16) ---
        def emit_tree_cg(vts, cg):
            # u = e[4cg]+e[4cg+2], e[4cg+1]+e[4cg+3]; v = u0+u1
            eng = nc.gpsimd if cg == 3 else nc.vector
            ut = small.tile([P, 2 * NW], BF16, tag="tu")
            eng.tensor_add(
                ut[:],
                _f3(ebuf[:, 4 * cg : 4 * cg + 2]),
                _f3(ebuf[:, 4 * cg + 2 : 4 * cg + 4]),
            )
            eng.tensor_add(vts[cg][:], ut[:, 0:NW], ut[:, NW : 2 * NW])

        def emit_s(vts, sball, rall, rb):
            # p-mul for each channel is deferred to the head of its conv
            # pass (next iteration's channel loop) so the DVE work spreads
            # across the PE's conv stream instead of bursting at the
            # iteration boundary.
            t01 = small.tile([P, NW], BF16, tag="t01")
            t23 = small.tile([P, NW], BF16, tag="t23")
            nc.vector.tensor_add(t01[:], vts[0][:], vts[1][:])
            nc.vector.tensor_add(t23[:], vts[2][:], vts[3][:])
            nc.vector.tensor_add(sball[:], t01[:], t23[:])  # f32 out
            nc.vector.reciprocal_approx_fast(rall[:], sball[:])
            nc.vector.tensor_scalar_mul(rb[:], rall[:], 1.0)  # f32 -> bf16

        def new_smax_tiles():
            sball = smax.tile([P, NW], F32, tag="S")
            rall = smax.tile([P, NW], F32, tag="r")
            rb = smax.tile([P, NW], BF16, tag="rb")
            vts = [smax.tile([P, NW], BF16, tag=f"tv{g}", name=f"vt{g}") for g in range(4)]
            return sball, rall, rb, vts

        # Prologue: softmax of iteration 0 from x0 (exp per 4-channel group).
        sball, rall, rb, vts = new_smax_tiles()
        for cg in range(4):
            nc.scalar.activation(
                out=_f3(ebuf[:, 4 * cg : 4 * cg + 4]),
                in_=_f3(x0b[:, 4 * cg : 4 * cg + 4]),
                func=AF.Exp,
            )
            emit_tree_cg(vts, cg)
        emit_s(vts, sball, rall, rb)

        for it in range(n_iter):
            last = it == n_iter - 1
            if not last:
                nball, nrall, nrb, nvts = new_smax_tiles()
            # ---- smoothing convs, per channel ----
            for c in range(C):
                # p = e * r for this channel (e left in ebuf by the previous
                # iteration's exp; r from its channel-sum).
                nc.vector.tensor_mul(
                    out=_f2(ebuf[:, c]), in0=_f2(ebuf[:, c]), in1=rb[:]
                )
                pA = psum.tile([P, NCH, 512], F32, tag="ps")
                for m in range(NCW):
                    for j in range(NCH):
                        # CoreSim needs j==0 to cover the full width (its
                        # pending-zero model can't mix accumulate/overwrite in
                        # one matmul); HW has_written handles the banded
                        # overlap per element.
                        n0, n1 = (0, H) if (j == 0 and full_j0) else _band(j, H)
                        nc.tensor.matmul(
                            pA[:, m, n0:n1],
                            lhsT=ebuf[:, c, j, m * P : (m + 1) * P],
                            rhs=th_sb[:, j, n0:n1],
                            start=(j == 0),
                            stop=(j == NCH - 1),
                        )
                o1 = stage.tile([P, NCW, H], BF16, tag="o1")
                if c % 2 == 0:
                    nc.scalar.copy(out=o1[:], in_=pA[:, :, 0:H])
                else:
                    nc.vector.tensor_scalar_mul(o1[:], pA[:, :, 0:H], 1.0)
                pB = psum.tile([P, NCH, 512], F32, tag="ps")
                for m in range(NCH):
                    if not last:
                        # Seed this PSUM bank with x0 (identity matmul); the
                        # W-conv accumulates on top so PSUM ends holding
                        # x = x0 + s. (One matmul per bank — a single MM's
                        # output cannot span PSUM banks.)
                        nc.tensor.matmul(
                            pB[:, m, 0:W],
                            lhsT=ident[:],
                            rhs=x0b[:, c, m],
                            start=True,
                            stop=False,
                        )
                    for j in range(NCW):
                        n0, n1 = (0, W) if (j == 0 and full_j0 and last) else _band(j, W)
                        nc.tensor.matmul(
                            pB[:, m, n0:n1],
                            lhsT=o1[:, j, m * P : (m + 1) * P],
                            rhs=tw_sb[:, j, n0:n1],
                            start=(j == 0 and last),
                            stop=(j == NCW - 1),
                        )
                if not last:
                    # e = exp(x) straight out of PSUM.
                    nc.scalar.activation(
                        out=ebuf[:, c], in_=pB[:, :, 0:W], func=AF.Exp
                    )
                    if c % 4 == 3:
                        emit_tree_cg(nvts, c // 4)
                else:
                    nc.vector.tensor_add(
                        out=xbuf[:, c], in0=x0b[:, c], in1=pB[:, :, 0:W]
                    )
            if not last:
                emit_s(nvts, nball, nrall, nrb)
                sball, rall, rb, vts = nball, nrall, nrb, nvts

        # ---- final log_softmax: out = x - log(sum_c exp(x)) ----
        lball = smax.tile([P, NW], F32, tag="r")
        fvts = [smax.tile([P, NW], BF16, tag=f"tv{g}", name=f"fvt{g}") for g in range(4)]
        for cg in range(4):
            nc.scalar.activation(
                out=_f3(ebuf[:, 4 * cg : 4 * cg + 4]),
                in_=_f3(xbuf[:, 4 * cg : 4 * cg + 4]),
                func=AF.Exp,
            )
            emit_tree_cg(fvts, cg)
        ft01 = small.tile([P, NW], BF16, tag="t01")
        ft23 = small.tile([P, NW], BF16, tag="t23")
        fS = smax.tile([P, NW], F32, tag="S")
        nc.vector.tensor_add(ft01[:], fvts[0][:], fvts[1][:])
        nc.vector.tensor_add(ft23[:], fvts[2][:], fvts[3][:])
        nc.vector.tensor_add(fS[:], ft01[:], ft23[:])
        nc.scalar.activation(out=lball[:], in_=fS[:], func=AF.Ln)
        lb_v = lball[:].rearrange("p (a b) -> p a b", a=NCH)
        for c in range(C):
            eng = nc.gpsimd if c % 4 == 3 else nc.vector
            eng.tensor_sub(out=xbuf[:, c], in0=xbuf[:, c], in1=lb_v)
        nc.sync.dma_start(
            out=out_d[b].rearrange("c (j p) w -> p c j w", p=P),
            in_=xbuf[:],
        )


def build_nc(n_samples=BPC, n_iter=N_ITER, full_j0=False):
    # Bacc (not plain Bass): its compile() pass legalizes multi-wait
    # instructions via InstEventSemaphore — walrus caps regular instructions
    # at ONE sync wait.
    nc = bacc.Bacc()
    x_in = nc.dram_tensor("x", [n_samples, C, H, W], BF16, kind="ExternalInput")
    th_in = nc.dram_tensor("th", [n_samples, H, H], BF16, kind="ExternalInput")
    tw_in = nc.dram_tensor("tw", [n_samples, W, W], BF16, kind="ExternalInput")
    id_in = nc.dram_tensor("ident", [P, P], BF16, kind="ExternalInput")
    out_d = nc.dram_tensor("out", [n_samples, C, H, W], F32, kind="ExternalOutput")
    with tile.TileContext(nc) as tc:
        with ExitStack() as ctx:
            _crf_kernel(
                ctx, tc, out_d, x_in, th_in, tw_in, id_in, n_samples, n_iter, full_j0
            )
    nc.finalize()
    return nc


def make_toeplitz(spacing, inv_theta, size, weight=1.0):
    """Banded symmetric Toeplitz matrix for the 1D 'same' correlation."""
    d = spacing * np.arange(-(FS // 2), FS // 2 + 1, dtype=np.float32)
    k = np.exp(-((d * inv_theta) ** 2) / 2.0).astype(np.float32)
    k[FS // 2] = 0.0
    t = np.zeros((size, size), dtype=np.float32)
    for tap in range(FS):
        off = tap - FS // 2  # out[h] += k[tap] * x[h + off]
        idx = np.arange(max(0, -off), min(size, size - off))
        t[idx + off, idx] = k[tap]
    return (t * weight).astype(np.float32)


def host_prep(x, spatial_spacings, smoothness_weight, inv_smoothness_theta):
    """Build per-sample Th (H-conv) and weight-scaled Tw (W-conv) matrices
    plus the bf16 copy of x; all conv-path operands ship as bf16."""
    import ml_dtypes

    w = float(np.asarray(smoothness_weight))
    th = np.stack(
        [
            make_toeplitz(float(spatial_spacings[b, 0]), float(inv_smoothness_theta[0]), H)
            for b in range(x.shape[0])
        ]
    ).astype(ml_dtypes.bfloat16)
    tw = np.stack(
        [
            make_toeplitz(
                float(spatial_spacings[b, 1]), float(inv_smoothness_theta[1]), W, weight=w
            )
            for b in range(x.shape[0])
        ]
    ).astype(ml_dtypes.bfloat16)
    xb = np.ascontiguousarray(x).astype(ml_dtypes.bfloat16)
    return xb, th, tw


def make_ident():
    import ml_dtypes

    return np.eye(P, dtype=np.float32).astype(ml_dtypes.bfloat16)


_NC_CACHE = {}


def kernel(x, spatial_spacings, smoothness_weight, inv_smoothness_theta):
    from concourse.bass_utils import run_bass_kernel_spmd

    x = np.ascontiguousarray(np.asarray(x), dtype=np.float32)
    spatial_spacings = np.asarray(spatial_spacings, dtype=np.float32)
    xb, th, tw = host_prep(x, spatial_spacings, smoothness_weight, inv_smoothness_theta)
    ident = make_ident()

    key = (BPC, N_ITER)
    if key not in _NC_CACHE:
        _NC_CACHE[key] = build_nc(BPC, N_ITER)
    nc = _NC_CACHE[key]

    core_ids = list(range(N_CORES))
    in_maps = []
    for i in core_ids:
        sl = slice(i * BPC, (i + 1) * BPC)
        in_maps.append({"x": xb[sl], "th": th[sl], "tw": tw[sl], "ident": ident})
    res = run_bass_kernel_spmd(nc, in_maps, core_ids)
    out = np.concatenate([res.results[i]["out"] for i in core_ids], axis=0)
    return out.astype(np.float32)


if __name__ == "__main__":
    rng = np.random.default_rng(0)
    x = rng.standard_normal((B, C, H, W), dtype=np.float32)
    out = kernel(
        x,
        np.ones((B, 2), np.float32),
        np.float32(1.0),
        np.ones((2,), np.float32),
    )
    print(out.shape, out.dtype)
